# revision 1
# baseline (speedup 1.0000x reference)
"""Distributed Trainium2 kernel for single-head attention with QKV projections.

Reference computation (B=4, N=4096, D=256, fp32):
    q = x @ Wq_w.T + Wq_b
    k = z @ Wk_w.T + Wk_b
    v = z @ Wv_w.T + Wv_b
    out = softmax(q @ k.T / sqrt(D)) @ v

Sharding: pure data-parallel over (batch, query-half) across 8 cores. Core c
handles batch b = c//2, query rows [h*2048, (h+1)*2048) with h = c%2, and holds
the full z[b] so K/V are recomputed per core (2x duplication of the tiny D*D
projections). No collectives.

Key algebraic fold: q.k = x Wq^T Wk z^T + (bq Wk).z + (x Wq^T bk + bq.bk).
The last group is constant per query row and cancels in softmax, so only
  M  = Wq^T Wk        (folded on host, applied to x on device)
  v2 = (bq Wk)/16     (per-key scalar, rides as an extra column of the V
                       projection and becomes the per-partition Exp bias)
survive — the entire K projection disappears; the scores matmul contracts
x M directly against raw z.

Layout: everything is kept "transposed" so no PE transposes are needed:
  - qM[dz, i] comes out of the M-projection with the feature dim on
    partitions, exactly the rhs layout the scores matmul wants; zT (already
    resident) is the lhsT.
  - scores are computed transposed, sT[j, i] (keys on partitions), so the
    exp'd probabilities are directly the lhsT of the PV matmul.
  - a ones-column appended to v makes the PV matmul also produce the softmax
    denominator; normalization + Wv_b bias-add are fused into one DVE op.
Compute is bf16 (PE at 1 cycle/row vs 4 for fp32), accumulation fp32 in PSUM.
Softmax skips max-subtraction: scores/sqrt(D) are ~N(0, 0.65) here, bounded
by ~+-4, so exp() is safe in fp32.

Constants are packed host-side into two [128, *] DRAM tensors (Wpack/Bpack):
DMA issue cost on the sequencer is ~5ns per descriptor ~= per partition-row,
so one wide transfer beats many narrow ones. A warm-up matmul burst bridges
the input-DMA window because the PE clock ramps 0.65 -> 2.4 GHz over ~4us of
continuous execution and resets on idle.
"""

import numpy as np
import ml_dtypes

B, N, D = 4, 4096, 256
NCORES = 8
S = N // 2          # query rows per core
P = 128             # partitions
QBLK = 512          # scores free-dim block (one PSUM bank)
NJT = N // P        # 32 key tiles
NQB = S // QBLK     # 4 query blocks per core
DC = D // P         # 2 chunks of the feature dim
NWARM = 12          # PE p-state warm-up matmuls (bridge until first input DMA)

BF16 = ml_dtypes.bfloat16

_CACHE = {}


def _build():
    import concourse.mybir as mybir
    import concourse.tile as tile
    from concourse import bacc

    bf16 = mybir.dt.bfloat16
    f32 = mybir.dt.float32
    AF = mybir.ActivationFunctionType
    ALU = mybir.AluOpType

    nc = bacc.Bacc("TRN2", target_bir_lowering=False, debug=False, num_devices=NCORES)

    # Wpack columns: [M chunk0 | M chunk1 | (v2|Wv^T) chunk0 | (v2|Wv^T) chunk1]
    WCOLS = 2 * D + 2 * (D + 1)
    # xT: [128, 2*S] = [chunk0 | chunk1]; zT: [128, 2*N] = [c0h0|c1h0|c0h1|c1h1]
    # (partition-dim chunks packed along the free dim: one DMA of [128, X]
    # costs 128 descriptors regardless of X, so packing halves issue time)
    xT = nc.dram_tensor("xT", [P, DC * S], bf16, kind="ExternalInput").ap()
    zT = nc.dram_tensor("zT", [P, DC * N], bf16, kind="ExternalInput").ap()
    Wpack = nc.dram_tensor("Wpack", [P, WCOLS], bf16, kind="ExternalInput").ap()
    Bpack = nc.dram_tensor("Bpack", [P, D], f32, kind="ExternalInput").ap()
    out = nc.dram_tensor("out", [S, D], f32, kind="ExternalOutput").ap()

    with tile.TileContext(nc) as tc:
        with (
            tc.tile_pool(name="consts", bufs=1) as cp,
            tc.tile_pool(name="big", bufs=1) as bp,
            tc.tile_pool(name="pblk", bufs=3) as pp,
            tc.tile_pool(name="outp", bufs=8) as op,
            tc.tile_pool(name="psum", bufs=4, space="PSUM") as ps,
        ):
            # ---- PE warm-up ----
            wrm = cp.tile([P, P + QBLK], bf16, tag="warm", name="warm")
            nc.vector.memset(wrm[:], 0.0)
            wps = ps.tile([P, QBLK], f32, tag="proj", name="warm_ps")
            for _ in range(NWARM):
                nc.tensor.matmul(
                    wps[:], wrm[:, 0:P], wrm[:, P:P + QBLK], start=True, stop=True
                )

            # ---- input / constant DMAs (ordered by first use) ----
            wpk = cp.tile([P, WCOLS], bf16, tag="wpk", name="wpk")
            nc.sync.dma_start(wpk[:], Wpack[:])
            xTp = bp.tile([P, DC * S], bf16, tag="xTp", name="xTp")
            zTp = bp.tile([P, DC * N], bf16, tag="zTp", name="zTp")
            nc.sync.dma_start(xTp[:], xT[:])
            for h in range(2):
                nc.sync.dma_start(
                    zTp[:, h * N:(h + 1) * N], zT[:, h * N:(h + 1) * N]
                )
            bpk = cp.tile([P, D], f32, tag="bpk", name="bpk")
            nc.sync.dma_start(bpk[:], Bpack[:])

            def xs(c, lo, hi):   # xT chunk c, query cols [lo, hi)
                return xTp[:, c * S + lo:c * S + hi]

            def zs(c, lo, hi):   # zT chunk c, key cols [lo, hi) (within a half)
                h, r = divmod(lo, N // 2)
                assert hi - lo <= N // 2 - r
                o = h * N + c * (N // 2) + r
                return zTp[:, o:o + (hi - lo)]

            def m_sl(c):   # M lhsT chunk c: [128, 256]
                return wpk[:, c * D:(c + 1) * D]

            def wv_sl(c):  # (v2 | Wv^T) rhs chunk c: [128, 257]
                o = 2 * D + c * (D + 1)
                return wpk[:, o:o + D + 1]

            bvb_sb = bpk[:, 0:D]

            # ---- qM projection: (x M)^T[dz, i] over [256, 2048] ----
            qM_sb = []
            for e in range(DC):
                t = bp.tile([P, S], bf16, tag=f"qM{e}", name=f"qM{e}")
                qM_sb.append(t)
            for jb in range(S // QBLK):
                for e in range(DC):
                    acc = ps.tile([P, QBLK], f32, tag="proj", name="proj_ps")
                    for c in range(DC):
                        nc.tensor.matmul(
                            acc[:],
                            m_sl(c)[:, e * P:(e + 1) * P],
                            xs(c, jb * QBLK, (jb + 1) * QBLK),
                            start=(c == 0),
                            stop=(c == DC - 1),
                        )
                    nc.scalar.activation(
                        qM_sb[e][:, jb * QBLK:(jb + 1) * QBLK], acc[:], AF.Copy
                    )

            # ---- v projection: [t3 | v | 1] per key tile ----
            # psum col 0 accumulates t3[j] = (bq Wk).z_j / 16 (the per-key
            # score bias), cols 1:257 the value rows; col 257 is set to 1 so
            # the PV matmul also produces the softmax denominator.
            # (single wide tiles instead of 32 small ones: every distinct
            # tile tag costs semaphores whose teardown clears dominate the
            # kernel tail at ~115ns each)
            VW = D + 2
            vbig = bp.tile([P, NJT * VW], bf16, tag="vbig", name="vbig")
            # one strided memset covers every ones-column
            nc.vector.memset(
                vbig[:].rearrange("p (t w) -> p t w", w=VW)[:, :, D + 1:D + 2], 1.0
            )
            for t_i in range(NJT):
                acc = ps.tile([P, D + 1], f32, tag="sc", name="v_ps")
                for c in range(DC):
                    nc.tensor.matmul(
                        acc[:],
                        zs(c, t_i * P, (t_i + 1) * P),
                        wv_sl(c),
                        start=(c == 0),
                        stop=(c == DC - 1),
                    )
                o = t_i * VW
                nc.vector.tensor_copy(vbig[:, o:o + D + 1], acc[:])

            # ---- attention, per query block of 512 ----
            for qb in range(NQB):
                ptb = pp.tile([P, NJT * QBLK], bf16, tag="pT", name="pT")
                for t_i in range(NJT):
                    acc = ps.tile([P, QBLK], f32, tag="sc", name="sc_ps")
                    for c in range(DC):
                        nc.tensor.matmul(
                            acc[:],
                            zs(c, t_i * P, (t_i + 1) * P),
                            qM_sb[c][:, qb * QBLK:(qb + 1) * QBLK],
                            start=(c == 0),
                            stop=(c == DC - 1),
                        )
                    pt = ptb[:, t_i * QBLK:(t_i + 1) * QBLK]
                    # p = exp(scores/16 + t3); the query-constant score terms
                    # are dropped — they cancel in the softmax normalization.
                    nc.scalar.activation(
                        pt[:], acc[:], AF.Exp,
                        bias=vbig[:, t_i * VW:t_i * VW + 1], scale=1.0 / 16.0,
                    )

                # PV accumulation with t outer so PE consumes exp'd tiles in
                # the order ACT produces them (no stall on the last exps).
                pvs = [
                    ps.tile([P, D + 1], f32, tag="proj", name=f"pv_ps{sq}")
                    for sq in range(QBLK // P)
                ]
                # t-outer so PE consumes exp'd tiles in production order;
                # the last 4 rounds run sq-major so the per-sq accumulations
                # finish staggered and the DVE normalize + output-DMA chains
                # overlap the remaining PV matmuls instead of all queueing
                # after the final one.
                def pt_sl(t_i, sq):
                    o = t_i * QBLK + sq * P
                    return ptb[:, o:o + P]

                def v_sl(t_i):
                    return vbig[:, t_i * VW + 1:t_i * VW + D + 2]

                for t_i in range(NJT - 4):
                    for sq in range(QBLK // P):
                        nc.tensor.matmul(
                            pvs[sq][:], pt_sl(t_i, sq), v_sl(t_i),
                            start=(t_i == 0), stop=False,
                        )
                for sq in range(QBLK // P):
                    for t_i in range(NJT - 4, NJT):
                        nc.tensor.matmul(
                            pvs[sq][:], pt_sl(t_i, sq), v_sl(t_i),
                            start=False, stop=(t_i == NJT - 1),
                        )
                for sq in range(QBLK // P):
                    pv = pvs[sq]
                    recip = op.tile([P, 1], f32, tag="recip", name="recip")
                    nc.vector.reciprocal(recip[:], pv[:, D:D + 1])
                    ot = op.tile([P, D], f32, tag="ot", name="ot")
                    # out = (pv * 1/denom) + bv
                    nc.vector.scalar_tensor_tensor(
                        ot[:], pv[:, 0:D], recip[:], bvb_sb,
                        op0=ALU.mult, op1=ALU.add,
                    )
                    r0 = (qb * (QBLK // P) + sq) * P
                    nc.sync.dma_start(out[r0:r0 + P, :], ot[:])

    nc.compile()
    return nc


def _get_nc():
    if "nc" not in _CACHE:
        _CACHE["nc"] = _build()
    return _CACHE["nc"]


def _prep_in_maps(x, z, Wq_w, Wq_b, Wk_w, Wk_b, Wv_w, Wv_b):
    x = np.asarray(x, np.float32)
    z = np.asarray(z, np.float32)
    Wq = np.asarray(Wq_w, np.float64)
    Wk = np.asarray(Wk_w, np.float64)
    bq = np.asarray(Wq_b, np.float64)

    M = (Wq.T @ Wk).astype(np.float32)           # [dx, dz]
    v2 = ((bq @ Wk) / 16.0).astype(np.float32)   # [dz]
    WvT = np.ascontiguousarray(np.asarray(Wv_w, np.float32).T)  # [dz, e]

    WCOLS = 2 * D + 2 * (D + 1)
    Wpack = np.empty((P, WCOLS), BF16)
    for c in range(DC):
        Wpack[:, c * D:(c + 1) * D] = M[c * P:(c + 1) * P, :].astype(BF16)
        o = 2 * D + c * (D + 1)
        Wpack[:, o] = v2[c * P:(c + 1) * P].astype(BF16)
        Wpack[:, o + 1:o + 1 + D] = WvT[c * P:(c + 1) * P, :].astype(BF16)
    Bpack = np.ascontiguousarray(
        np.broadcast_to(np.asarray(Wv_b, np.float32), (P, D))
    )

    in_maps = []
    for core in range(NCORES):
        b, h = divmod(core, 2)
        xTc = np.ascontiguousarray(x[b].T[:, h * S:(h + 1) * S]).astype(BF16)
        zTc = np.ascontiguousarray(z[b].T).astype(BF16)
        xTp = np.hstack([xTc[0:P], xTc[P:2 * P]])
        zTp = np.hstack([
            zTc[0:P, 0:N // 2], zTc[P:2 * P, 0:N // 2],
            zTc[0:P, N // 2:N], zTc[P:2 * P, N // 2:N],
        ])
        in_maps.append({
            "xT": xTp, "zT": zTp,
            "Wpack": Wpack, "Bpack": Bpack,
        })
    return in_maps


def kernel(x, z, Wq_w, Wq_b, Wk_w, Wk_b, Wv_w, Wv_b):
    from concourse.bass_utils import run_bass_kernel_spmd

    in_maps = _prep_in_maps(x, z, Wq_w, Wq_b, Wk_w, Wk_b, Wv_w, Wv_b)
    nc = _get_nc()
    _CACHE["in_maps"] = in_maps
    res = run_bass_kernel_spmd(nc, in_maps, core_ids=list(range(NCORES)))

    full = np.empty((B, N, D), np.float32)
    for core in range(NCORES):
        b, h = divmod(core, 2)
        full[b, h * S:(h + 1) * S, :] = res.results[core]["out"]
    return full



# revision 3
# speedup vs baseline: 1.2197x; 1.2197x over previous
"""Distributed Trainium2 kernel for single-head attention with QKV projections.

Reference computation (B=4, N=4096, D=256, fp32):
    q = x @ Wq_w.T + Wq_b
    k = z @ Wk_w.T + Wk_b
    v = z @ Wv_w.T + Wv_b
    out = softmax(q @ k.T / sqrt(D)) @ v

Sharding: pure data-parallel over (batch, query-half) across 8 cores. Core c
handles batch b = c//2, query rows [h*2048, (h+1)*2048) with h = c%2, and holds
the full z[b] so K/V are recomputed per core. No collectives.

Algebra: q.k = x Wq^T Wk z^T + (bq Wk).z + const(query), and the query-constant
terms cancel in softmax. With M = Wq^T Wk folded on host:
    scores = (x M + 1.(bq Wk)) z^T          — one matrix, bias folded into the
                                               projection output
so the K projection disappears entirely and the per-key score bias rides as a
per-partition bias on the qM-projection copy (ACT Identity+bias), not as an
extra matmul column.

fp8 scores path: the N^2 D scores matmul runs in fp8e4 (e4m3) with
perf_mode=DoubleRow, which packs both 128-feature chunks into one virtual
256-row matmul at ~1.5x bf16 throughput. z ships as fp8 from the host
(tile-major [p, t*256 + c*128 + j] so the [Ki,Ko=2,M] lhsT AP is a simple
slice+rearrange); qM is cast to fp8 by the projection's ACT copy. Measured
end-to-end rel-err ~1.5e-2 (gate 2e-2) — dominated by the fp8 quantization
noise on both scores operands; PV stays bf16 to protect precision.

Layout: everything is kept "transposed" so no PE transposes are needed:
  - qM8[dz, i] comes out of the projection with the feature dim on partitions,
    exactly the rhs layout the scores matmul wants; zT8 is the lhsT.
  - scores are computed transposed, sT[j, i] (keys on partitions), so the
    exp'd probabilities are directly the lhsT of the PV matmul.
  - a ones-column appended to v makes the PV matmul also produce the softmax
    denominator; normalization + Wv_b bias-add fuse into one DVE op.

exp runs on ACT in [128, 1024] instructions (a 2-bank PSUM tile holding two
key-tiles' scores) — the ~370ns fixed SBUF/PSUM access latency per ACT
instruction amortizes over 1024 columns instead of 512. This is only possible
because the bias fold removed the per-key-tile exp bias.

The attention loop is software-pipelined BY EMIT ORDER (engine queues are
strict FIFO): scores of query-block qb interleave with PV of qb-1 at pair
granularity, so the PE never sits behind the ACT exp tail of its own block.
PV consumes exp'd tiles in production order; the last 4 tiles run sq-major so
the per-sq accumulations finish staggered and normalize+DMA overlap the
remaining matmuls. Output DMA is one batched [128, 4, 256] descriptor set per
query block.

A warm-up accumulation group (back-to-back matmuls into one PSUM bank) bridges
the input-DMA window because the PE clock ramps 0.65 -> 2.4 GHz over ~3us of
continuous execution and resets on idle.
"""

import numpy as np
import ml_dtypes

B, N, D = 4, 4096, 256
NCORES = 8
S = N // 2          # query rows per core
P = 128             # partitions
QBLK = 512          # scores free-dim block (one PSUM bank)
NJT = N // P        # 32 key tiles
NQB = S // QBLK     # 4 query blocks per core
DC = D // P         # 2 chunks of the feature dim
NWARM = 10          # PE p-state warm-up matmuls (bridge until first input DMA)
VW = D + 1          # v tile width: [v | 1]

BF16 = ml_dtypes.bfloat16
FP8 = ml_dtypes.float8_e4m3

_CACHE = {}


def _build():
    import concourse.mybir as mybir
    import concourse.tile as tile
    from concourse import bacc

    bf16 = mybir.dt.bfloat16
    fp8 = mybir.dt.float8e4
    f32 = mybir.dt.float32
    AF = mybir.ActivationFunctionType
    ALU = mybir.AluOpType
    DR = mybir.MatmulPerfMode.DoubleRow

    nc = bacc.Bacc("TRN2", target_bir_lowering=False, debug=False, num_devices=NCORES)

    # xT: [128, 2*S] = [chunk0 | chunk1] bf16 (proj rhs)
    # zT8: [128, NJT*256] fp8, tile-major: [p, t*256 + c*128 + j] (scores lhsT)
    # zT: [128, 2*N] bf16 = [c0h0|c1h0|c0h1|c1h1] (v-proj lhsT)
    # Wpack cols: [M chunk0 | M chunk1 | WvT chunk0 | WvT chunk1] bf16
    # Bpack: [128, D+2] f32 = [Wv_b broadcast | bqWk chunk0 | bqWk chunk1]
    xT = nc.dram_tensor("xT", [P, DC * S], bf16, kind="ExternalInput").ap()
    zT8 = nc.dram_tensor("zT8", [P, NJT * D], fp8, kind="ExternalInput").ap()
    zT = nc.dram_tensor("zT", [P, DC * N], bf16, kind="ExternalInput").ap()
    Wpack = nc.dram_tensor("Wpack", [P, 4 * D], bf16, kind="ExternalInput").ap()
    Bpack = nc.dram_tensor("Bpack", [P, D + 2], f32, kind="ExternalInput").ap()
    out = nc.dram_tensor("out", [S, D], f32, kind="ExternalOutput").ap()

    with tile.TileContext(nc) as tc:
        with (
            tc.tile_pool(name="consts", bufs=1) as cp,
            tc.tile_pool(name="big", bufs=1) as bp,
            tc.tile_pool(name="pblk", bufs=2) as pp,
            tc.tile_pool(name="outp", bufs=8) as op,
            tc.tile_pool(name="psum", bufs=1, space="PSUM") as ps,
        ):
            # ---- PE warm-up: one accumulation group, back-to-back ----
            wrm = cp.tile([P, P + QBLK], bf16, tag="warm", name="warm")
            nc.vector.memset(wrm[:], 0.0)
            wps = ps.tile([P, 2 * QBLK], f32, tag="sc", bufs=2, name="warm_ps")
            for i in range(NWARM):
                nc.tensor.matmul(
                    wps[:, 0:QBLK], wrm[:, 0:P], wrm[:, P:P + QBLK],
                    start=(i == 0), stop=(i == NWARM - 1),
                )

            # ---- input / constant DMAs (ordered by first use) ----
            wpk = cp.tile([P, 4 * D], bf16, tag="wpk", name="wpk")
            nc.sync.dma_start(wpk[:], Wpack[:])
            xTp = bp.tile([P, DC * S], bf16, tag="xTp", name="xTp")
            nc.sync.dma_start(xTp[:], xT[:])
            bpk = cp.tile([P, D + 2], f32, tag="bpk", name="bpk")
            nc.sync.dma_start(bpk[:], Bpack[:])
            zT8p = bp.tile([P, NJT * D], fp8, tag="zT8p", name="zT8p")
            for h in range(2):
                nc.sync.dma_start(
                    zT8p[:, h * (NJT * D // 2):(h + 1) * (NJT * D // 2)],
                    zT8[:, h * (NJT * D // 2):(h + 1) * (NJT * D // 2)],
                )
            zTp = bp.tile([P, DC * N], bf16, tag="zTp", name="zTp")
            for h in range(2):
                nc.sync.dma_start(
                    zTp[:, h * N:(h + 1) * N], zT[:, h * N:(h + 1) * N]
                )

            def xs(c, lo, hi):   # xT chunk c, query cols [lo, hi)
                return xTp[:, c * S + lo:c * S + hi]

            def zs(c, lo, hi):   # zT chunk c, key cols [lo, hi) (within a half)
                h, r = divmod(lo, N // 2)
                assert hi - lo <= N // 2 - r
                o = h * N + c * (N // 2) + r
                return zTp[:, o:o + (hi - lo)]

            def zs8(t_i):        # fp8 scores lhsT for key tile t_i: [128, 2, 128]
                return zT8p[:, t_i * D:(t_i + 1) * D].rearrange(
                    "p (c j) -> p c j", c=DC
                )

            def m_sl(c):   # M lhsT chunk c: [128, 256]
                return wpk[:, c * D:(c + 1) * D]

            def wv_sl(c):  # WvT rhs chunk c: [128, 256]
                return wpk[:, (DC + c) * D:(DC + c + 1) * D]

            bvb_sb = bpk[:, 0:D]

            # ---- qM projection -> fp8: (x M + bqWk)^T[dz, i] over [256, 2048]
            qM8 = bp.tile([P, DC * S], fp8, tag="qM8", name="qM8")
            qM8r = qM8[:].rearrange("p (c i) -> p c i", c=DC)
            for jb in range(S // QBLK):
                for e in range(DC):
                    acc = ps.tile([P, QBLK], f32, tag="pv", bufs=4, name="proj_ps")
                    for c in range(DC):
                        nc.tensor.matmul(
                            acc[:],
                            m_sl(c)[:, e * P:(e + 1) * P],
                            xs(c, jb * QBLK, (jb + 1) * QBLK),
                            start=(c == 0),
                            stop=(c == DC - 1),
                        )
                    nc.scalar.activation(
                        qM8[:, e * S + jb * QBLK:e * S + (jb + 1) * QBLK],
                        acc[:], AF.Identity,
                        bias=bpk[:, D + e:D + e + 1],
                    )

            # ---- v projection: [v | 1] per key tile, bf16 ----
            vbig = bp.tile([P, NJT * VW], bf16, tag="vbig", name="vbig")
            nc.vector.memset(
                vbig[:].rearrange("p (t w) -> p t w", w=VW)[:, :, D:D + 1], 1.0
            )
            for t_i in range(NJT):
                acc = ps.tile([P, D], f32, tag="pv", bufs=4, name="v_ps")
                for c in range(DC):
                    nc.tensor.matmul(
                        acc[:],
                        zs(c, t_i * P, (t_i + 1) * P),
                        wv_sl(c),
                        start=(c == 0),
                        stop=(c == DC - 1),
                    )
                nc.vector.tensor_copy(vbig[:, t_i * VW:t_i * VW + D], acc[:])

            def v_sl(t_i):
                return vbig[:, t_i * VW:t_i * VW + D + 1]

            # ---- attention, software-pipelined: scores(qb) || PV(qb-1) ----
            ptbs = [None, None]

            def emit_scores_pair(qb, u):
                st = ps.tile([P, 2 * QBLK], f32, tag="sc", bufs=2, name="sc_ps")
                for k in range(2):
                    nc.tensor.matmul(
                        st[:, k * QBLK:(k + 1) * QBLK],
                        zs8(2 * u + k),
                        qM8r[:, :, qb * QBLK:(qb + 1) * QBLK],
                        start=True, stop=True, perf_mode=DR,
                    )
                ptb = ptbs[qb % 2]
                nc.scalar.activation(
                    ptb[:, 2 * u * QBLK:(2 * u + 2) * QBLK], st[:],
                    AF.Exp, scale=1.0 / 16.0,
                )

            def emit_pv(pb, pvs, t_i, sq):
                ptb = ptbs[pb % 2]
                o = t_i * QBLK + sq * P
                nc.tensor.matmul(
                    pvs[sq][:], ptb[:, o:o + P], v_sl(t_i),
                    start=(t_i == 0), stop=(t_i == NJT - 1),
                )

            def emit_drain(pb, pvs, sq):
                pv = pvs[sq]
                recip = op.tile([P, 1], f32, tag="recip", name="recip")
                nc.vector.reciprocal(recip[:], pv[:, D:D + 1])
                ob = obufs[pb % 2]
                nc.vector.scalar_tensor_tensor(
                    ob[:, sq * D:(sq + 1) * D], pv[:, 0:D], recip[:], bvb_sb,
                    op0=ALU.mult, op1=ALU.add,
                )

            obufs = [
                bp.tile([P, 4 * D], f32, tag=f"ob{i}", name=f"ob{i}")
                for i in range(2)
            ]

            pvs_cur = None
            for qb in range(NQB + 1):
                if qb < NQB:
                    ptbs[qb % 2] = pp.tile(
                        [P, NJT * QBLK], bf16, tag="pT", name="pT"
                    )
                if qb >= 1:
                    pvs_cur = [
                        ps.tile([P, D + 1], f32, tag="pv", bufs=4,
                                name=f"pv_ps{sq}")
                        for sq in range(QBLK // P)
                    ]
                for u in range(NJT // 2):
                    if qb < NQB:
                        emit_scores_pair(qb, u)
                    if qb >= 1:
                        # PV of block qb-1, tiles 2u, 2u+1; last 4 tiles
                        # sq-major so accumulations finish staggered.
                        if u < NJT // 2 - 2:
                            for t_i in (2 * u, 2 * u + 1):
                                for sq in range(QBLK // P):
                                    emit_pv(qb - 1, pvs_cur, t_i, sq)
                        elif u == NJT // 2 - 2:
                            pass  # deferred to the sq-major tail below
                if qb >= 1:
                    for sq in range(QBLK // P):
                        for t_i in range(NJT - 4, NJT):
                            emit_pv(qb - 1, pvs_cur, t_i, sq)
                        emit_drain(qb - 1, pvs_cur, sq)
                    ob = obufs[(qb - 1) % 2]
                    dst = out[(qb - 1) * 4 * P:qb * 4 * P, :].rearrange(
                        "(s p) e -> p s e", p=P
                    )
                    nc.sync.dma_start(
                        dst, ob[:].rearrange("p (s e) -> p s e", s=4)
                    )

    nc.compile()
    return nc


def _get_nc():
    if "nc" not in _CACHE:
        _CACHE["nc"] = _build()
    return _CACHE["nc"]


def _prep_in_maps(x, z, Wq_w, Wq_b, Wk_w, Wk_b, Wv_w, Wv_b):
    x = np.asarray(x, np.float32)
    z = np.asarray(z, np.float32)
    Wq = np.asarray(Wq_w, np.float64)
    Wk = np.asarray(Wk_w, np.float64)
    bq = np.asarray(Wq_b, np.float64)

    M = (Wq.T @ Wk).astype(np.float32)           # [dx, dz]
    bqWk = (bq @ Wk).astype(np.float32)          # [dz]
    WvT = np.ascontiguousarray(np.asarray(Wv_w, np.float32).T)  # [dz, e]

    Wpack = np.empty((P, 4 * D), BF16)
    for c in range(DC):
        Wpack[:, c * D:(c + 1) * D] = M[c * P:(c + 1) * P, :].astype(BF16)
        Wpack[:, (DC + c) * D:(DC + c + 1) * D] = WvT[c * P:(c + 1) * P, :].astype(BF16)
    Bpack = np.empty((P, D + 2), np.float32)
    Bpack[:, 0:D] = np.broadcast_to(np.asarray(Wv_b, np.float32), (P, D))
    for c in range(DC):
        Bpack[:, D + c] = bqWk[c * P:(c + 1) * P]

    in_maps = []
    for core in range(NCORES):
        b, h = divmod(core, 2)
        xTc = np.ascontiguousarray(x[b].T[:, h * S:(h + 1) * S]).astype(BF16)
        xTp = np.hstack([xTc[0:P], xTc[P:2 * P]])
        zTc = np.ascontiguousarray(z[b].T)
        zTb = zTc.astype(BF16)
        zTp = np.hstack([
            zTb[0:P, 0:N // 2], zTb[P:2 * P, 0:N // 2],
            zTb[0:P, N // 2:N], zTb[P:2 * P, N // 2:N],
        ])
        # fp8 z, tile-major: [p, t*256 + c*128 + j] = z[t*128+j, c*128+p]
        z8 = zTc.astype(FP8).reshape(DC, P, NJT, P)        # [c, p, t, j]
        zT8p = np.ascontiguousarray(
            z8.transpose(1, 2, 0, 3).reshape(P, NJT * D)
        )
        in_maps.append({
            "xT": xTp, "zT8": zT8p, "zT": zTp,
            "Wpack": Wpack, "Bpack": Bpack,
        })
    return in_maps


def kernel(x, z, Wq_w, Wq_b, Wk_w, Wk_b, Wv_w, Wv_b):
    from concourse.bass_utils import run_bass_kernel_spmd

    in_maps = _prep_in_maps(x, z, Wq_w, Wq_b, Wk_w, Wk_b, Wv_w, Wv_b)
    nc = _get_nc()
    _CACHE["in_maps"] = in_maps
    res = run_bass_kernel_spmd(nc, in_maps, core_ids=list(range(NCORES)))

    full = np.empty((B, N, D), np.float32)
    for core in range(NCORES):
        b, h = divmod(core, 2)
        full[b, h * S:(h + 1) * S, :] = res.results[core]["out"]
    return full
